# revision 13
# baseline (speedup 1.0000x reference)
"""Trainium2 Bass kernel for a ResNet Bottleneck block (inference), fp8.

Reference computation (NCHW, N=128, Cin=Cout=1024, width=256, H=W=14):
    out = relu(bn1(conv1x1(x, w1)))          # 1024 -> 256
    out = relu(bn2(conv3x3(out, w2, pad=1))) # 256 -> 256
    out = bn3(conv1x1(out, w3))              # 256 -> 1024
    y   = relu(out + x)

Strategy (fp8 e4m3 DoubleRow):
- Data-parallel: 16 images per NeuronCore (8 cores), params replicated.
- BN folded on host into weight scale + bias. All convs run on the PE in
  fp8 (TRN e4m3, max 240) with MatmulPerfMode.DoubleRow: each matmul
  contracts TWO 128-row K-slices per pass (2x bf16 MACs/cycle; measured
  166ns for K=256,N=392 - same as one bf16 K=128,N=392 matmul).
- Scales keep everything in e4m3's sweet range and evictions cheap:
    xq = fp8(x); w1q = fp8(w1*2^5)  -> psum1 = 2^5*conv1
    out1 = fp8(relu(psum1 + 2^5 b1))               [no multiply needed]
    w2q = fp8(w2*2^2)               -> psum2 = 2^7*conv2
    out2 = fp8(relu(0.5*psum2 + 2^6 b2))           [ACT scale op]
    w3q = fp8(w3*2^10)              -> psum3 = 2^16*conv3
    y = relu(psum3*2^-16 + (x+b3))                 [DVE stt + relu]
  Measured end-to-end rel err ~7.5e-3 (gate 2e-2).
- conv2 (3x3, pad 1) uses zero-padded 16x16 per-image fields and the
  flat-window trick: tap (dy,dx) is one shifted 512-wide window over two
  images; invalid rows/cols land outside the evicted 14x14 subview.
- conv3 is interleaved with conv2 at image-pair (np) granularity so the
  DVE residual-add (scalar_tensor_tensor) work overlaps PE work instead
  of piling up at the end. Program order: c2(np0), c2(np1), c3(np0),
  c2(np2), c3(np1), ... so ACT evictions of c2(np) hide under c2(np+1).
- Eviction engine split per np block: conv2 on ACT (needs scale), stt on
  DVE, final relu 3/8 ACT + 5/8 Pool. Residual x+b3 ships bf16 np-major
  so the first conv3 block's slice arrives right after the fp8 x.
"""

import sys

if "/opt/trn_rl_repo" not in sys.path:
    sys.path.insert(0, "/opt/trn_rl_repo")

import numpy as np
import ml_dtypes

import concourse.bass as bass
import concourse.bacc as bacc
import concourse.tile as tile
from concourse import mybir
from concourse.bass_utils import run_bass_kernel_spmd

EPS = 1e-5
NCORES = 8
NLOC = 16          # images per core
C_IN = 1024
WIDTH = 256
C_OUT = 1024
HW = 196           # 14*14
P = 128
KB1 = C_IN // P    # 8 input k-blocks
KB2 = WIDTH // P   # 2
MB3 = C_OUT // P   # 8 output m-blocks
NF = 2 * HW        # 392 (one image pair)
PAD = 256          # 16x16 padded field per image

A1, A2, A3 = 5, 2, 10   # host weight scales (powers of 2)
C1, C2 = 5, 6           # storage scales of out1 / out2
SC2 = 2.0 ** (C2 - C1 - A2)     # conv2 eviction scale (0.5)
SC3 = 2.0 ** (-(C2 + A3))       # conv3 eviction scale (2^-16)

FP8 = ml_dtypes.float8_e4m3     # TRN e4m3 flavor: max normal 240
BF16H = ml_dtypes.bfloat16
F8 = mybir.dt.float8e4
BF16 = mybir.dt.bfloat16
F32 = mybir.dt.float32
Relu = mybir.ActivationFunctionType.Relu
DR = mybir.MatmulPerfMode.DoubleRow

NP_ORDER = [4, 5, 6, 7, 0, 1, 2, 3]   # half 1 first (see conv2 spill note)

_cached = {}


def _build():
    if "nc" in _cached:
        return _cached["nc"]

    nc = bacc.Bacc("TRN2", target_bir_lowering=False, debug=False,
                   num_devices=NCORES)

    # x fp8: [half, k, p, 8 images * 196]
    xq_d = nc.dram_tensor("xq", [2, KB1, P, NLOC * HW // 2], F8,
                          kind="ExternalInput")
    # residual x+b3, bf16, np-major: [half, npl, k, p, 392]
    xr_d = nc.dram_tensor("xr", [2, 4, KB1, P, NF], BF16,
                          kind="ExternalInput")
    # weights pre-arranged as SBUF images (partition-major)
    w1_d = nc.dram_tensor("w1t", [P, 4 * KB2 * WIDTH], F8,
                          kind="ExternalInput")
    w2_d = nc.dram_tensor("w2t", [P, 9 * KB2 * WIDTH], F8,
                          kind="ExternalInput")
    w3_d = nc.dram_tensor("w3t", [P, KB2 * C_OUT], F8, kind="ExternalInput")
    b_d = nc.dram_tensor("biases", [P, 2 * KB2], F32, kind="ExternalInput")
    y_d = nc.dram_tensor("y", [MB3, P, NLOC * HW], BF16,
                         kind="ExternalOutput")

    with tile.TileContext(nc) as tc:
        _emit(tc, nc, xq_d, xr_d, w1_d, w2_d, w3_d, b_d, y_d)

    nc.compile()
    _cached["nc"] = nc
    return nc


def _emit(tc, nc, xq_d, xr_d, w1_d, w2_d, w3_d, b_d, y_d):
    import contextlib

    Alu = mybir.AluOpType
    from concourse.tile import add_dep_helper

    with contextlib.ExitStack() as ctx:
        const = ctx.enter_context(tc.tile_pool(name="const", bufs=1))
        xpool = ctx.enter_context(tc.tile_pool(name="xpool", bufs=1))
        opool = ctx.enter_context(tc.tile_pool(name="opool", bufs=1))
        psp = ctx.enter_context(tc.tile_pool(name="psp", bufs=8, space="PSUM"))
        evp = ctx.enter_context(tc.tile_pool(name="evp", bufs=2))

        # ---- input loads ------------------------------------------------
        # xq sbuf [P, k, half, 1568]; h0 split in 2-k chunks chained
        # depth-1 so conv1's first k-pair lands ASAP (transfers that run
        # concurrently stripe across DMA engines and all finish late)
        xq = xpool.tile([P, KB1 * NLOC * HW], F8, name="xq", tag="xq")
        xqv = xq[:].rearrange("p (k h c) -> p k h c", k=KB1, h=2)
        x_dmas = []
        chunks = [(0, 0, 4), (0, 4, 8), (1, 0, 4), (1, 4, 8)]
        for half, k0, k1 in chunks:
            dst = xqv[:, k0:k1, half, :]
            src = (xq_d.ap()[half][k0:k1].rearrange("k p c -> p k c"))
            i = nc.sync.dma_start(dst, src)
            if len(x_dmas) >= 2:
                add_dep_helper(i.ins, x_dmas[-2], reason="x load pacing")
            x_dmas.append(i.ins)

        w1sb = const.tile([P, 4 * KB2 * WIDTH], F8, name="w1sb", tag="w1sb")
        nc.scalar.dma_start(w1sb[:], w1_d.ap())
        w1v = w1sb[:].rearrange("p (j k c) -> p j k c", j=4, k=KB2)

        ball = const.tile([P, 2 * KB2], F32, name="ball", tag="ball")
        i = nc.scalar.dma_start(ball[:], b_d.ap())
        add_dep_helper(i.ins, x_dmas[0], reason="bias after early x")
        b1_t = ball[:, 0:KB2]
        b2_t = ball[:, KB2:2 * KB2]

        # w2/w3 issue from the scalar queue (gpsimd must stay clear for the
        # early memsets; a chained dma_start blocks its issuing queue)
        w2sb = const.tile([P, 9 * KB2 * WIDTH], F8, name="w2sb", tag="w2sb")
        i = nc.scalar.dma_start(w2sb[:], w2_d.ap())
        add_dep_helper(i.ins, x_dmas[2], reason="w2 after most x")
        w2v = w2sb[:].rearrange("p (t k c) -> p t k c", t=9, k=KB2)

        w3sb = const.tile([P, KB2 * C_OUT], F8, name="w3sb", tag="w3sb")
        i = nc.scalar.dma_start(w3sb[:], w3_d.ap())
        add_dep_helper(i.ins, x_dmas[3], reason="w3 after x")
        w3v = w3sb[:].rearrange("p (k c) -> p k c", k=KB2)

        # residual (x + b3) bf16, np-major; one DMA per np, in NP_ORDER,
        # issued from sync (idle early) so gpsimd/scalar stay unblocked
        xr = xpool.tile([P, KB1 * NLOC * HW], BF16, name="xr", tag="xr")
        xrv = xr[:].rearrange("p (k h l c) -> p k h l c", k=KB1, h=2, l=4)
        prev = x_dmas[3]
        xr_dmas = {}
        for np_ in NP_ORDER:
            h, l = np_ // 4, np_ % 4
            dst = xrv[:, :, h, l, :]
            src = xr_d.ap()[h][l].rearrange("k p c -> p k c")
            i = nc.sync.dma_start(dst, src)
            add_dep_helper(i.ins, prev, reason="xr pacing")
            prev = i.ins
            xr_dmas[np_] = i.ins

        # ---- PE warm-up (p-state ramp needs ~3.4us of activity) ---------
        scratch = const.tile([P, 512], BF16, name="scratch", tag="scratch")
        nc.gpsimd.memset(scratch[:], 0.0)
        warm_ps = psp.tile([P, 512], F32, name="warm_ps", tag="ps")
        for _ in range(8):
            nc.tensor.matmul(warm_ps[:], scratch[:, 0:P], scratch[:],
                             start=True, stop=True)

        # ---- conv1 output: zero-padded fields with SHARED pad rows ------
        # per k-block: images at 15-row (240-elem) stride; image i's field
        # is rows [15i, 15i+16) - its bottom pad row is image i+1's top
        # pad. 241 rows total = 3856 elems + 64 tail for window spill
        # (max base (14*15+2)*16+2 = 3394, +480 = 3874 <= 3920).
        IST = 15 * 16                                     # image stride 240
        KSTR = NLOC * IST + 16 + 64                       # 3920
        o1 = opool.tile([P, KB2 * KSTR], F8, name="o1", tag="o1")
        o1k = o1[:].rearrange("p (k c) -> p k c", k=KB2)  # [P, 2, 3920]
        o1kirc = o1k[:, :, 0:NLOC * IST].rearrange(
            "p k (i r c) -> p k i r c", i=NLOC, r=15)
        nc.gpsimd.memset(o1kirc[:, :, :, 0, :], 0.0)      # top pad rows
        nc.gpsimd.memset(o1kirc[:, :, :, :, 0], 0.0)      # left pad cols
        nc.gpsimd.memset(o1kirc[:, :, :, :, 15], 0.0)     # right pad cols
        nc.gpsimd.memset(o1k[:, :, NLOC * IST:], 0.0)     # last row + tail

        out2 = opool.tile([P, KB2 * NLOC * HW], F8, name="out2", tag="out2")
        o2v = out2[:].rearrange("p (k c) -> p k c", k=KB2)

        # ---- conv1 (1x1, 1024->256) + bias + relu -> padded o1 ----------
        # DoubleRow: 4 k-pair steps. Per half: 8 open psum groups.
        for half in range(2):
            nps = [half * 4 + j for j in range(4)]
            grp = {}
            for np_ in nps:
                for m in range(KB2):
                    grp[(np_, m)] = psp.tile([P, NF], F32,
                                             name=f"ps1_{np_}_{m}", tag="ps")
            for j in range(4):
                for m in range(KB2):
                    for np_ in nps:
                        rhs = xqv[:, 2 * j:2 * j + 2, half,
                                  (np_ % 4) * NF:(np_ % 4 + 1) * NF]
                        nc.tensor.matmul(
                            grp[(np_, m)][:],
                            w1v[:, j, :, m * P:(m + 1) * P],
                            rhs, start=(j == 0), stop=(j == 3),
                            perf_mode=DR)
            for np_ in nps:
                for m in range(KB2):
                    dst = o1kirc[:, m, 2 * np_:2 * np_ + 2, 1:15, 1:15]
                    # (p, i2, 14, 14) after k collapses
                    src = (grp[(np_, m)][:]
                           .rearrange("p (i r c) -> p i r c", i=2, r=14))
                    # h0: alternate ACT/DVE (serializing on one engine
                    # stalls h1's psum-bank reuse); h1: all DVE so ACT is
                    # clear for ev_c2(first np) which gates conv3
                    if half == 0 and np_ % 2 == 0:
                        nc.scalar.activation(dst, src, Relu,
                                             bias=b1_t[:, m:m + 1])
                    else:
                        nc.vector.tensor_scalar(dst, src, b1_t[:, m:m + 1],
                                                0.0, Alu.add, Alu.max)

        # ---- conv2 + conv3, interleaved per np block --------------------
        # conv2: 9 DoubleRow taps over flat 512-wide windows of o1.
        # conv3: 8 single DoubleRow matmuls (one per output m-block).
        def c2_alloc(np_):
            return [psp.tile([P, 2 * IST], F32, name=f"ps2_{np_}_{m}",
                             tag="ps") for m in range(KB2)]

        def c2_tap(np_, g, tap):
            dy, dx = tap // 3, tap % 3
            base = (2 * np_ * 15 + dy) * 16 + dx
            for m in range(KB2):
                nc.tensor.matmul(
                    g[m][:], w2v[:, tap, :, m * P:(m + 1) * P],
                    o1k[:, :, base:base + 2 * IST],
                    start=(tap == 0), stop=(tap == 8), perf_mode=DR)

        def evict_conv2(np_, g):
            for m in range(KB2):
                src = (g[m][:].rearrange("p (i r c) -> p i r c", i=2, r=15)
                       [:, :, 0:14, 0:14])
                dst = (o2v[:, m, np_ * NF:(np_ + 1) * NF]
                       .rearrange("p (i r c) -> p i r c", i=2, r=14))
                nc.scalar.activation(dst, src, Relu, bias=b2_t[:, m:m + 1],
                                     scale=SC2)

        def c3_one(np_, tsum, m):
            # one conv3 matmul + its DVE residual stt, emitted inline so
            # PSUM banks free at the DVE cadence instead of piling up
            ps = psp.tile([P, NF], F32, name=f"ps3_{np_}_{m}", tag="ps")
            nc.tensor.matmul(
                ps[:], w3v[:, :, m * P:(m + 1) * P],
                o2v[:, :, np_ * NF:(np_ + 1) * NF],
                start=True, stop=True, perf_mode=DR)
            h, l = np_ // 4, np_ % 4
            nc.vector.scalar_tensor_tensor(
                tsum[:, m * NF:(m + 1) * NF], ps[:], SC3,
                xrv[:, m, h, l, :], Alu.mult, Alu.add)

        def c3_finish(np_, tsum, ystage, m0, m1):
            # merged relu (ACT; per-op fixed cost amortizes; Pool tensor
            # ops are ~5.7us each on GpSimd - never put evictions there)
            nc.scalar.activation(ystage[:, m0 * NF:m1 * NF],
                                 tsum[:, m0 * NF:m1 * NF], Relu, bias=0.0)
            nc.gpsimd.dma_start(
                y_d.ap()[m0:m1, :, np_ * NF:(np_ + 1) * NF]
                .rearrange("m p c -> p m c"),
                ystage[:, m0 * NF:m1 * NF]
                .rearrange("p (m c) -> p m c", m=m1 - m0))

        # Interleave: emit conv2(np+1) taps with conv3(np) matmuls so the
        # PE never waits on a long run of conv3 PSUM banks (freed at the
        # slower DVE stt cadence).
        order = NP_ORDER
        g2 = c2_alloc(order[0])
        for tap in range(9):
            c2_tap(order[0], g2, tap)
        for idx in range(len(order)):
            np_ = order[idx]
            tsum = evp.tile([P, MB3 * NF], BF16, name="tsum", tag="tsum",
                            bufs=4)
            ystage = evp.tile([P, MB3 * NF], BF16, name="ystage",
                              tag="ystage", bufs=4)
            if idx + 1 < len(order):
                nxt = order[idx + 1]
                gn = c2_alloc(nxt)
                c2_tap(nxt, gn, 0)
                c2_tap(nxt, gn, 1)
                evict_conv2(np_, g2)             # ACT, under next taps
                c2_tap(nxt, gn, 2)
                for m in range(MB3):             # taps 3..8 interleaved
                    if m < 6:
                        c2_tap(nxt, gn, 3 + m)
                    c3_one(np_, tsum, m)
                g2 = gn
                c3_finish(np_, tsum, ystage, 0, MB3)
            else:
                # tail block: split finish halves so relu/DMA overlap the
                # trailing stt chain
                evict_conv2(np_, g2)
                for m in range(MB3):
                    c3_one(np_, tsum, m)
                    if m in (1, 3, 5):
                        c3_finish(np_, tsum, ystage, m - 1, m + 1)
                c3_finish(np_, tsum, ystage, 6, MB3)


def _prep(x, w1, g1, b1, m1, v1, w2, g2, b2, m2, v2, w3, g3, b3, m3, v3):
    """Host-side: fold BN, scale + quantize to fp8, shard x."""
    def fold(w, g, b, m, v):
        scale = (g.astype(np.float64) / np.sqrt(v.astype(np.float64) + EPS))
        bias = b.astype(np.float64) - m.astype(np.float64) * scale
        wf = w.astype(np.float64) * scale.reshape(-1, *([1] * (w.ndim - 1)))
        return wf, bias

    def q8(a):
        return np.clip(a, -240.0, 240.0).astype(np.float32).astype(FP8)

    w1f, bias1 = fold(w1, g1, b1, m1, v1)   # [256,1024,1,1]
    w2f, bias2 = fold(w2, g2, b2, m2, v2)   # [256,256,3,3]
    w3f, bias3 = fold(w3, g3, b3, m3, v3)   # [1024,256,1,1]

    # w1t [p, j(4), kk(2), co(256)]
    w1q = q8(w1f[:, :, 0, 0].T.reshape(4, 2, P, WIDTH).transpose(2, 0, 1, 3)
             .reshape(P, 4 * 2 * WIDTH) * 2.0 ** A1)
    # w2t [p, tap(9), kk(2), co(256)], tap = dy*3+dx
    w2q = q8(w2f.transpose(2, 3, 1, 0).reshape(9, 2, P, WIDTH)
             .transpose(2, 0, 1, 3).reshape(P, 9 * 2 * WIDTH) * 2.0 ** A2)
    # w3t [p, kk(2), co(1024)]
    w3q = q8(w3f[:, :, 0, 0].T.reshape(2, P, C_OUT).transpose(1, 0, 2)
             .reshape(P, 2 * C_OUT) * 2.0 ** A3)

    b1h = (bias1 * 2.0 ** C1).reshape(KB2, P).T       # [P, 2]
    b2h = (bias2 * 2.0 ** C2).reshape(KB2, P).T       # [P, 2]
    ball = np.ascontiguousarray(
        np.concatenate([b1h, b2h], axis=1), dtype=np.float32)

    # xq fp8: [core, half, k, p, 8*196]
    x6 = x.reshape(NCORES, 2, 8, KB1, P, HW)          # c, half, img, k, p, hw
    xq = q8(np.ascontiguousarray(x6.transpose(0, 1, 3, 4, 2, 5))
            .reshape(NCORES, 2, KB1, P, NLOC * HW // 2))

    # residual x+b3 bf16, np-major: [core, half, npl, k, p, 392]
    xb = (x.astype(np.float64)
          + bias3.reshape(1, C_OUT, 1, 1)).astype(np.float32)
    x7 = xb.reshape(NCORES, 2, 4, 2, KB1, P, HW)      # c,h,npl,i,k,p,hw
    xr = np.ascontiguousarray(x7.transpose(0, 1, 2, 4, 5, 3, 6)
                              .reshape(NCORES, 2, 4, KB1, P, NF)
                              ).astype(BF16H)

    common = {"w1t": w1q, "w2t": w2q, "w3t": w3q, "biases": ball}
    in_maps = [dict(common, xq=np.ascontiguousarray(xq[i]),
                    xr=np.ascontiguousarray(xr[i]))
               for i in range(NCORES)]
    return in_maps


def kernel(**inputs):
    inputs = {k: np.asarray(v) for k, v in inputs.items()}
    in_maps = _prep(**inputs)
    nc = _build()
    res = run_bass_kernel_spmd(nc, in_maps, core_ids=list(range(NCORES)))

    y = np.empty((NCORES * NLOC, C_OUT, 14, 14), dtype=np.float32)
    for i in range(NCORES):
        r = np.asarray(res.results[i]["y"], dtype=np.float32)  # [m,P,3136]
        r = (r.reshape(MB3, P, 2, 8, HW)
             .transpose(2, 3, 0, 1, 4)
             .reshape(NLOC, C_OUT, 14, 14))
        y[i * NLOC:(i + 1) * NLOC] = r
    return y


# revision 14
# speedup vs baseline: 1.0505x; 1.0505x over previous
"""Trainium2 Bass kernel for a ResNet Bottleneck block (inference), fp8.

Reference computation (NCHW, N=128, Cin=Cout=1024, width=256, H=W=14):
    out = relu(bn1(conv1x1(x, w1)))          # 1024 -> 256
    out = relu(bn2(conv3x3(out, w2, pad=1))) # 256 -> 256
    out = bn3(conv1x1(out, w3))              # 256 -> 1024
    y   = relu(out + x)

Strategy (fp8 e4m3 DoubleRow):
- Data-parallel: 16 images per NeuronCore (8 cores), params replicated.
- BN folded on host into weight scale + bias. All convs run on the PE in
  fp8 (TRN e4m3, max 240) with MatmulPerfMode.DoubleRow: each matmul
  contracts TWO 128-row K-slices per pass (2x bf16 MACs/cycle; measured
  166ns for K=256,N=392 - same as one bf16 K=128,N=392 matmul).
- Scales keep everything in e4m3's sweet range and evictions cheap:
    xq = fp8(x); w1q = fp8(w1*2^5)  -> psum1 = 2^5*conv1
    out1 = fp8(relu(psum1 + 2^5 b1))               [no multiply needed]
    w2q = fp8(w2*2^2)               -> psum2 = 2^7*conv2
    out2 = fp8(relu(0.5*psum2 + 2^6 b2))           [ACT scale op]
    w3q = fp8(w3*2^10)              -> psum3 = 2^16*conv3
    y = relu(psum3*2^-16 + (x+b3))                 [DVE stt + relu]
  Measured end-to-end rel err ~7.5e-3 (gate 2e-2).
- conv2 (3x3, pad 1) uses zero-padded 16x16 per-image fields and the
  flat-window trick: tap (dy,dx) is one shifted 512-wide window over two
  images; invalid rows/cols land outside the evicted 14x14 subview.
- conv3 is interleaved with conv2 at image-pair (np) granularity so the
  DVE residual-add (scalar_tensor_tensor) work overlaps PE work instead
  of piling up at the end. Program order: c2(np0), c2(np1), c3(np0),
  c2(np2), c3(np1), ... so ACT evictions of c2(np) hide under c2(np+1).
- Eviction engine split per np block: conv2 on ACT (needs scale), stt on
  DVE, final relu 3/8 ACT + 5/8 Pool. Residual x+b3 ships bf16 np-major
  so the first conv3 block's slice arrives right after the fp8 x.
"""

import sys

if "/opt/trn_rl_repo" not in sys.path:
    sys.path.insert(0, "/opt/trn_rl_repo")

import numpy as np
import ml_dtypes

import concourse.bass as bass
import concourse.bacc as bacc
import concourse.tile as tile
from concourse import mybir
from concourse.bass_utils import run_bass_kernel_spmd

EPS = 1e-5
NCORES = 8
NLOC = 16          # images per core
C_IN = 1024
WIDTH = 256
C_OUT = 1024
HW = 196           # 14*14
P = 128
KB1 = C_IN // P    # 8 input k-blocks
KB2 = WIDTH // P   # 2
MB3 = C_OUT // P   # 8 output m-blocks
NF = 2 * HW        # 392 (one image pair)
PAD = 256          # 16x16 padded field per image

A1, A2, A3 = 5, 2, 10   # host weight scales (powers of 2)
C1, C2 = 5, 6           # storage scales of out1 / out2
SC2 = 2.0 ** (C2 - C1 - A2)     # conv2 eviction scale (0.5)
SC3 = 2.0 ** (-(C2 + A3))       # conv3 eviction scale (2^-16)

FP8 = ml_dtypes.float8_e4m3     # TRN e4m3 flavor: max normal 240
BF16H = ml_dtypes.bfloat16
F8 = mybir.dt.float8e4
BF16 = mybir.dt.bfloat16
F32 = mybir.dt.float32
Relu = mybir.ActivationFunctionType.Relu
DR = mybir.MatmulPerfMode.DoubleRow

NP_ORDER = [4, 5, 6, 7, 0, 1, 2, 3]   # half 1 first (see conv2 spill note)

_cached = {}


def _build():
    if "nc" in _cached:
        return _cached["nc"]

    nc = bacc.Bacc("TRN2", target_bir_lowering=False, debug=False,
                   num_devices=NCORES)

    # x fp8: [half, k, p, 8 images * 196]
    xq_d = nc.dram_tensor("xq", [2, KB1, P, NLOC * HW // 2], F8,
                          kind="ExternalInput")
    # residual x+b3, bf16, np-major: [half, npl, k, p, 392]
    xr_d = nc.dram_tensor("xr", [2, 4, KB1, P, NF], BF16,
                          kind="ExternalInput")
    # weights pre-arranged as SBUF images (partition-major)
    w1_d = nc.dram_tensor("w1t", [P, 4 * KB2 * WIDTH], F8,
                          kind="ExternalInput")
    w2_d = nc.dram_tensor("w2t", [P, 9 * KB2 * WIDTH], F8,
                          kind="ExternalInput")
    w3_d = nc.dram_tensor("w3t", [P, KB2 * C_OUT], F8, kind="ExternalInput")
    b_d = nc.dram_tensor("biases", [P, 2 * KB2], F32, kind="ExternalInput")
    y_d = nc.dram_tensor("y", [MB3, P, NLOC * HW], BF16,
                         kind="ExternalOutput")

    with tile.TileContext(nc) as tc:
        _emit(tc, nc, xq_d, xr_d, w1_d, w2_d, w3_d, b_d, y_d)

    nc.compile()
    _cached["nc"] = nc
    return nc


def _emit(tc, nc, xq_d, xr_d, w1_d, w2_d, w3_d, b_d, y_d):
    import contextlib

    Alu = mybir.AluOpType
    from concourse.tile import add_dep_helper

    with contextlib.ExitStack() as ctx:
        const = ctx.enter_context(tc.tile_pool(name="const", bufs=1))
        xpool = ctx.enter_context(tc.tile_pool(name="xpool", bufs=1))
        opool = ctx.enter_context(tc.tile_pool(name="opool", bufs=1))
        psp = ctx.enter_context(tc.tile_pool(name="psp", bufs=8, space="PSUM"))
        evp = ctx.enter_context(tc.tile_pool(name="evp", bufs=2))

        # ---- input loads ------------------------------------------------
        # xq sbuf [P, k, half, 1568]; h0 split in 2-k chunks chained
        # depth-1 so conv1's first k-pair lands ASAP (transfers that run
        # concurrently stripe across DMA engines and all finish late)
        xq = xpool.tile([P, KB1 * NLOC * HW], F8, name="xq", tag="xq")
        xqv = xq[:].rearrange("p (k h c) -> p k h c", k=KB1, h=2)
        x_dmas = []
        chunks = [(0, 0, 4), (0, 4, 8), (1, 0, 4), (1, 4, 8)]
        for half, k0, k1 in chunks:
            dst = xqv[:, k0:k1, half, :]
            src = (xq_d.ap()[half][k0:k1].rearrange("k p c -> p k c"))
            i = nc.sync.dma_start(dst, src)
            if len(x_dmas) >= 2:
                add_dep_helper(i.ins, x_dmas[-2], reason="x load pacing")
            x_dmas.append(i.ins)

        w1sb = const.tile([P, 4 * KB2 * WIDTH], F8, name="w1sb", tag="w1sb")
        nc.scalar.dma_start(w1sb[:], w1_d.ap())
        w1v = w1sb[:].rearrange("p (j k c) -> p j k c", j=4, k=KB2)

        ball = const.tile([P, 2 * KB2], F32, name="ball", tag="ball")
        i = nc.scalar.dma_start(ball[:], b_d.ap())
        add_dep_helper(i.ins, x_dmas[0], reason="bias after early x")
        b1_t = ball[:, 0:KB2]
        b2_t = ball[:, KB2:2 * KB2]

        # w2/w3 issue from the scalar queue (gpsimd must stay clear for the
        # early memsets; a chained dma_start blocks its issuing queue)
        w2sb = const.tile([P, 9 * KB2 * WIDTH], F8, name="w2sb", tag="w2sb")
        i = nc.scalar.dma_start(w2sb[:], w2_d.ap())
        add_dep_helper(i.ins, x_dmas[2], reason="w2 after most x")
        w2v = w2sb[:].rearrange("p (t k c) -> p t k c", t=9, k=KB2)

        w3sb = const.tile([P, KB2 * C_OUT], F8, name="w3sb", tag="w3sb")
        i = nc.scalar.dma_start(w3sb[:], w3_d.ap())
        add_dep_helper(i.ins, x_dmas[3], reason="w3 after x")
        w3v = w3sb[:].rearrange("p (k c) -> p k c", k=KB2)

        # residual (x + b3) bf16, np-major; one DMA per np, in NP_ORDER,
        # issued from sync (idle early) so gpsimd/scalar stay unblocked
        xr = xpool.tile([P, KB1 * NLOC * HW], BF16, name="xr", tag="xr")
        xrv = xr[:].rearrange("p (k h l c) -> p k h l c", k=KB1, h=2, l=4)
        prev = x_dmas[3]
        xr_dmas = {}
        for np_ in NP_ORDER:
            h, l = np_ // 4, np_ % 4
            dst = xrv[:, :, h, l, :]
            src = xr_d.ap()[h][l].rearrange("k p c -> p k c")
            i = nc.sync.dma_start(dst, src)
            add_dep_helper(i.ins, prev, reason="xr pacing")
            prev = i.ins
            xr_dmas[np_] = i.ins

        # ---- PE warm-up (p-state ramp needs ~3.4us of activity) ---------
        scratch = const.tile([P, 512], BF16, name="scratch", tag="scratch")
        nc.gpsimd.memset(scratch[:], 0.0)
        warm_ps = psp.tile([P, 512], F32, name="warm_ps", tag="ps")
        for _ in range(8):
            nc.tensor.matmul(warm_ps[:], scratch[:, 0:P], scratch[:],
                             start=True, stop=True)

        # ---- conv1 output: zero-padded fields with SHARED pad rows ------
        # per k-block: images at 15-row (240-elem) stride; image i's field
        # is rows [15i, 15i+16) - its bottom pad row is image i+1's top
        # pad. 241 rows total = 3856 elems + 64 tail for window spill
        # (max base (14*15+2)*16+2 = 3394, +480 = 3874 <= 3920).
        IST = 15 * 16                                     # image stride 240
        KSTR = NLOC * IST + 16 + 64                       # 3920
        o1 = opool.tile([P, KB2 * KSTR], F8, name="o1", tag="o1")
        o1k = o1[:].rearrange("p (k c) -> p k c", k=KB2)  # [P, 2, 3920]
        o1kirc = o1k[:, :, 0:NLOC * IST].rearrange(
            "p k (i r c) -> p k i r c", i=NLOC, r=15)
        nc.gpsimd.memset(o1kirc[:, :, :, 0, :], 0.0)      # top pad rows
        nc.gpsimd.memset(o1kirc[:, :, :, :, 0], 0.0)      # left pad cols
        nc.gpsimd.memset(o1kirc[:, :, :, :, 15], 0.0)     # right pad cols
        nc.gpsimd.memset(o1k[:, :, NLOC * IST:], 0.0)     # last row + tail

        out2 = opool.tile([P, KB2 * NLOC * HW], F8, name="out2", tag="out2")
        o2v = out2[:].rearrange("p (k c) -> p k c", k=KB2)

        # ---- conv1 (1x1, 1024->256) + bias + relu -> padded o1 ----------
        # DoubleRow: 4 k-pair steps. Per half: 8 open psum groups.
        for half in range(2):
            nps = [half * 4 + j for j in range(4)]
            grp = {}
            for np_ in nps:
                for m in range(KB2):
                    grp[(np_, m)] = psp.tile([P, NF], F32,
                                             name=f"ps1_{np_}_{m}", tag="ps")
            for j in range(4):
                for m in range(KB2):
                    for np_ in nps:
                        rhs = xqv[:, 2 * j:2 * j + 2, half,
                                  (np_ % 4) * NF:(np_ % 4 + 1) * NF]
                        nc.tensor.matmul(
                            grp[(np_, m)][:],
                            w1v[:, j, :, m * P:(m + 1) * P],
                            rhs, start=(j == 0), stop=(j == 3),
                            perf_mode=DR)
            for np_ in nps:
                for m in range(KB2):
                    dst = o1kirc[:, m, 2 * np_:2 * np_ + 2, 1:15, 1:15]
                    # (p, i2, 14, 14) after k collapses
                    src = (grp[(np_, m)][:]
                           .rearrange("p (i r c) -> p i r c", i=2, r=14))
                    # alternate ACT/DVE; serializing either engine stalls
                    # the next phase (psum-bank reuse / ev_c2 gating)
                    if np_ % 2 == 0:
                        nc.scalar.activation(dst, src, Relu,
                                             bias=b1_t[:, m:m + 1])
                    else:
                        nc.vector.tensor_scalar(dst, src, b1_t[:, m:m + 1],
                                                0.0, Alu.add, Alu.max)

        # ---- conv2 + conv3, interleaved per np block --------------------
        # conv2: 9 DoubleRow taps over flat 512-wide windows of o1.
        # conv3: 8 single DoubleRow matmuls (one per output m-block).
        def c2_alloc(np_):
            return [psp.tile([P, 2 * IST], F32, name=f"ps2_{np_}_{m}",
                             tag="ps") for m in range(KB2)]

        def c2_tap(np_, g, tap):
            dy, dx = tap // 3, tap % 3
            base = (2 * np_ * 15 + dy) * 16 + dx
            for m in range(KB2):
                nc.tensor.matmul(
                    g[m][:], w2v[:, tap, :, m * P:(m + 1) * P],
                    o1k[:, :, base:base + 2 * IST],
                    start=(tap == 0), stop=(tap == 8), perf_mode=DR)

        def evict_conv2(np_, g):
            for m in range(KB2):
                src = (g[m][:].rearrange("p (i r c) -> p i r c", i=2, r=15)
                       [:, :, 0:14, 0:14])
                dst = (o2v[:, m, np_ * NF:(np_ + 1) * NF]
                       .rearrange("p (i r c) -> p i r c", i=2, r=14))
                nc.scalar.activation(dst, src, Relu, bias=b2_t[:, m:m + 1],
                                     scale=SC2)

        def c3_one(np_, tsum, m):
            # one conv3 matmul + its DVE residual stt, emitted inline so
            # PSUM banks free at the DVE cadence instead of piling up
            ps = psp.tile([P, NF], F32, name=f"ps3_{np_}_{m}", tag="ps")
            nc.tensor.matmul(
                ps[:], w3v[:, :, m * P:(m + 1) * P],
                o2v[:, :, np_ * NF:(np_ + 1) * NF],
                start=True, stop=True, perf_mode=DR)
            h, l = np_ // 4, np_ % 4
            nc.vector.scalar_tensor_tensor(
                tsum[:, m * NF:(m + 1) * NF], ps[:], SC3,
                xrv[:, m, h, l, :], Alu.mult, Alu.add)

        def c3_finish(np_, tsum, ystage, m0, m1, on_vector=False):
            # merged relu (ACT; per-op fixed cost amortizes; Pool tensor
            # ops are ~5.7us each on GpSimd - never put evictions there)
            if on_vector:
                nc.vector.tensor_scalar(ystage[:, m0 * NF:m1 * NF],
                                        tsum[:, m0 * NF:m1 * NF],
                                        0.0, None, Alu.max)
            else:
                nc.scalar.activation(ystage[:, m0 * NF:m1 * NF],
                                     tsum[:, m0 * NF:m1 * NF], Relu,
                                     bias=0.0)
            nc.gpsimd.dma_start(
                y_d.ap()[m0:m1, :, np_ * NF:(np_ + 1) * NF]
                .rearrange("m p c -> p m c"),
                ystage[:, m0 * NF:m1 * NF]
                .rearrange("p (m c) -> p m c", m=m1 - m0))

        # Interleave: emit conv2(np+1) taps with conv3(np) matmuls so the
        # PE never waits on a long run of conv3 PSUM banks (freed at the
        # slower DVE stt cadence).
        order = NP_ORDER
        g2 = c2_alloc(order[0])
        for tap in range(9):
            c2_tap(order[0], g2, tap)
        for idx in range(len(order)):
            np_ = order[idx]
            tsum = evp.tile([P, MB3 * NF], BF16, name="tsum", tag="tsum",
                            bufs=4)
            ystage = evp.tile([P, MB3 * NF], BF16, name="ystage",
                              tag="ystage", bufs=4)
            if idx + 1 < len(order):
                nxt = order[idx + 1]
                gn = c2_alloc(nxt)
                c2_tap(nxt, gn, 0)
                c2_tap(nxt, gn, 1)
                evict_conv2(np_, g2)             # ACT, under next taps
                c2_tap(nxt, gn, 2)
                c2_tap(nxt, gn, 3)
                for m in range(MB3):             # taps 4..8 interleaved
                    if m < 5:
                        c2_tap(nxt, gn, 4 + m)
                    c3_one(np_, tsum, m)
                g2 = gn
                c3_finish(np_, tsum, ystage, 0, MB3)
            else:
                # tail block: split finish halves so relu/DMA overlap the
                # trailing stt chain
                evict_conv2(np_, g2)
                for m in range(MB3):
                    c3_one(np_, tsum, m)
                    if m in (1, 3, 5):
                        c3_finish(np_, tsum, ystage, m - 1, m + 1,
                                  on_vector=(m == 3))
                c3_finish(np_, tsum, ystage, 6, MB3, on_vector=True)


def _prep(x, w1, g1, b1, m1, v1, w2, g2, b2, m2, v2, w3, g3, b3, m3, v3):
    """Host-side: fold BN, scale + quantize to fp8, shard x."""
    def fold(w, g, b, m, v):
        scale = (g.astype(np.float64) / np.sqrt(v.astype(np.float64) + EPS))
        bias = b.astype(np.float64) - m.astype(np.float64) * scale
        wf = w.astype(np.float64) * scale.reshape(-1, *([1] * (w.ndim - 1)))
        return wf, bias

    def q8(a):
        return np.clip(a, -240.0, 240.0).astype(np.float32).astype(FP8)

    w1f, bias1 = fold(w1, g1, b1, m1, v1)   # [256,1024,1,1]
    w2f, bias2 = fold(w2, g2, b2, m2, v2)   # [256,256,3,3]
    w3f, bias3 = fold(w3, g3, b3, m3, v3)   # [1024,256,1,1]

    # w1t [p, j(4), kk(2), co(256)]
    w1q = q8(w1f[:, :, 0, 0].T.reshape(4, 2, P, WIDTH).transpose(2, 0, 1, 3)
             .reshape(P, 4 * 2 * WIDTH) * 2.0 ** A1)
    # w2t [p, tap(9), kk(2), co(256)], tap = dy*3+dx
    w2q = q8(w2f.transpose(2, 3, 1, 0).reshape(9, 2, P, WIDTH)
             .transpose(2, 0, 1, 3).reshape(P, 9 * 2 * WIDTH) * 2.0 ** A2)
    # w3t [p, kk(2), co(1024)]
    w3q = q8(w3f[:, :, 0, 0].T.reshape(2, P, C_OUT).transpose(1, 0, 2)
             .reshape(P, 2 * C_OUT) * 2.0 ** A3)

    b1h = (bias1 * 2.0 ** C1).reshape(KB2, P).T       # [P, 2]
    b2h = (bias2 * 2.0 ** C2).reshape(KB2, P).T       # [P, 2]
    ball = np.ascontiguousarray(
        np.concatenate([b1h, b2h], axis=1), dtype=np.float32)

    # xq fp8: [core, half, k, p, 8*196]
    x6 = x.reshape(NCORES, 2, 8, KB1, P, HW)          # c, half, img, k, p, hw
    xq = q8(np.ascontiguousarray(x6.transpose(0, 1, 3, 4, 2, 5))
            .reshape(NCORES, 2, KB1, P, NLOC * HW // 2))

    # residual x+b3 bf16, np-major: [core, half, npl, k, p, 392]
    xb = (x.astype(np.float64)
          + bias3.reshape(1, C_OUT, 1, 1)).astype(np.float32)
    x7 = xb.reshape(NCORES, 2, 4, 2, KB1, P, HW)      # c,h,npl,i,k,p,hw
    xr = np.ascontiguousarray(x7.transpose(0, 1, 2, 4, 5, 3, 6)
                              .reshape(NCORES, 2, 4, KB1, P, NF)
                              ).astype(BF16H)

    common = {"w1t": w1q, "w2t": w2q, "w3t": w3q, "biases": ball}
    in_maps = [dict(common, xq=np.ascontiguousarray(xq[i]),
                    xr=np.ascontiguousarray(xr[i]))
               for i in range(NCORES)]
    return in_maps


def kernel(**inputs):
    inputs = {k: np.asarray(v) for k, v in inputs.items()}
    in_maps = _prep(**inputs)
    nc = _build()
    res = run_bass_kernel_spmd(nc, in_maps, core_ids=list(range(NCORES)))

    y = np.empty((NCORES * NLOC, C_OUT, 14, 14), dtype=np.float32)
    for i in range(NCORES):
        r = np.asarray(res.results[i]["y"], dtype=np.float32)  # [m,P,3136]
        r = (r.reshape(MB3, P, 2, 8, HW)
             .transpose(2, 3, 0, 1, 4)
             .reshape(NLOC, C_OUT, 14, 14))
        y[i * NLOC:(i + 1) * NLOC] = r
    return y
